# revision 11
# baseline (speedup 1.0000x reference)
"""TRN2 Bass kernel for nn_CNN_2_EDropout: CNN forward + excitation-backprop
dropout. Data-parallel over 8 NeuronCores (64 samples each). All matmuls in
float32r (full-rate fp32-reduced mode, ~2e-5 rel err).

Layouts (per core, 64 samples):
- conv1 input: host-side im2col R [75=(ky,kx,ci), 64, 900=(30y,30x)]
- conv activations h1/h2 live in SBUF "block" layout [C, n, H, W] where
  H = out_h + 2, W = out_w + 2 with one zero pad row/col on each side; the
  required +-2 conv halo is served by the neighbouring sample's zero pad row,
  so shifted-offset matmuls use a single (rows, x) 3D access pattern whose
  32-row groups never cross PSUM bank boundaries.
- FC/EB stage keeps activations n-major [64, 2048]; contraction-major copies
  (h5T etc.) are built with PE transposes.
"""
import sys
import numpy as np

sys.path.insert(0, '/opt/trn_rl_repo')

import concourse.bass as bass  # noqa: E402
import concourse.tile as tile  # noqa: E402
from concourse import bacc, mybir  # noqa: E402
from concourse.bass import AP  # noqa: E402
from concourse.bass_utils import run_bass_kernel_spmd  # noqa: E402
from concourse.masks import make_identity  # noqa: E402
from concourse.alu_op_type import AluOpType  # noqa: E402

F32 = mybir.dt.float32
F32R = mybir.dt.float32r

NCORES = 8
BC = 64          # samples per core
P_KEEP = 0.5

# conv1: 3ch 32x32, k5 pad1 -> 30x30, pool -> 14x14
# conv2: 96 -> 128, k5 pad2 on 14x14 -> 14x14, pool -> 6x6
# conv3: 128 -> 256, k5 pad2 on 6x6 -> 6x6, pool -> 2x2
H1H, H1W = 16, 16      # h1 block: 1 zero pad row/col each side of 14x14
H1BLK = H1H * H1W      # 256
H2H, H2W = 8, 8        # h2 block: 1 pad around 6x6
H2BLK = H2H * H2W      # 64
LEAD = 32              # zeroed margin elems before/after block arrays

OFFS = [(dy, dx) for dy in range(-2, 3) for dx in range(-2, 3)]


def _xrange(dx, w_out):
    """Valid out-x range [x0, x0+cnt) for conv offset dx with 1-col padding."""
    x0 = 1 if dx == -2 else 0
    x1 = w_out - 1 if dx == 2 else w_out
    return x0, x1 - x0


def mkap(base: AP, off: int, dims):
    """AP at base.offset+off with free dims `dims` ([[step, count], ...]),
    keeping the partition dim of `base`."""
    return AP(tensor=base.tensor, offset=base.offset + off,
              ap=[list(base.ap[0])] + [list(d) for d in dims])


def build_bass(rep: int = 1):
    nc = bacc.Bacc("TRN2", target_bir_lowering=False, debug=False,
                   num_devices=NCORES)
    d = {}

    def din(name, shape, dt=F32R):
        d[name] = nc.dram_tensor(name, list(shape), dt, kind="ExternalInput").ap()

    din("r1", [75, BC * 900])
    din("w1t", [75, 96])
    din("w2t", [96, 25, 128])
    din("w3t", [128, 25, 256])
    din("fc1wt", [1024, 2048])
    din("fc2wt", [2048, 2048])
    din("wp2", [2048, 2048])
    din("fc3wt", [128, 16, 10])
    din("b1", [96], F32)
    din("b2", [128], F32)
    din("b3", [256], F32)
    din("fb1", [2048], F32)
    din("fb2", [2048], F32)
    din("fb3", [10], F32)
    din("g", [BC, 2048], F32)
    din("noise", [BC, 2048], F32)
    din("zeros", [LEAD + BC * H1BLK + LEAD])
    out_d = nc.dram_tensor("out", [BC, 10], F32, kind="ExternalOutput").ap()
    import os
    dbg = {}
    if os.environ.get("DBG_DUMP", "0") == "1":
        for nm in ("h5", "h6eb", "z2", "s2", "pebs", "retain", "mask", "h_ed",
                   "h6"):
            dbg[nm] = nc.dram_tensor("dbg_" + nm, [BC, 2048], F32,
                                     kind="ExternalOutput").ap()
        dbg["h4"] = nc.dram_tensor("dbg_h4", [128, 8, 64], F32,
                                   kind="ExternalOutput").ap()

    tc = tile.TileContext(nc, num_cores=NCORES)
    with tc:
        with tc.tile_pool(name="perm", bufs=1) as perm:
            # persistent across reps: biases, identity
            ident = perm.tile([64, 64], F32)
            make_identity(nc, ident)
            b1 = perm.tile([96, 1], F32)
            nc.sync.dma_start(out=b1, in_=d["b1"].rearrange("(c o) -> c o", o=1))
            b2 = perm.tile([128, 1], F32)
            nc.sync.dma_start(out=b2, in_=d["b2"].rearrange("(c o) -> c o", o=1))
            b3 = perm.tile([256 // 2, 2], F32)  # [co%128, tile] per M-tile
            nc.sync.dma_start(
                out=b3, in_=d["b3"].rearrange("(t c) -> c t", t=2))
            # fc bias rows replicated to 64 partitions
            fb1 = perm.tile([BC, 2048], F32)
            nc.gpsimd.dma_start(out=fb1, in_=AP(
                tensor=d["fb1"].tensor, offset=0, ap=[[0, BC], [1, 2048]]))
            fb2 = perm.tile([BC, 2048], F32)
            nc.gpsimd.dma_start(out=fb2, in_=AP(
                tensor=d["fb2"].tensor, offset=0, ap=[[0, BC], [1, 2048]]))
            fb3 = perm.tile([BC, 10], F32)
            nc.gpsimd.dma_start(out=fb3, in_=AP(
                tensor=d["fb3"].tensor, offset=0, ap=[[0, BC], [1, 10]]))
            gt = perm.tile([BC, 2048], F32)
            nc.sync.dma_start(out=gt, in_=d["g"])
            noise = perm.tile([BC, 2048], F32)
            nc.sync.dma_start(out=noise, in_=d["noise"])

            for _ in range(rep):
                one_pass(nc, tc, d, out_d, ident, b1, b2, b3, fb1, fb2, fb3,
                         gt, noise, dbg)
    nc.finalize()
    return nc


def one_pass(nc, tc, d, out_d, ident, b1, b2, b3, fb1, fb2, fb3, gt, noise,
             dbg):
    ctx_args = dict(nc=nc, tc=tc)

    with tc.tile_pool(name="acts", bufs=1) as acts:
        h3a = acts.tile([128, BC, 4], F32)
        h3b = acts.tile([128, BC, 4], F32)
        with tc.tile_pool(name="ph2", bufs=1) as ph2:
            h2 = ph2.tile([128, LEAD + BC * H2BLK + LEAD], F32R)
            nc.gpsimd.dma_start(out=h2, in_=AP(
                tensor=d["zeros"].tensor, offset=0,
                ap=[[0, 128], [1, LEAD + BC * H2BLK + LEAD]]))
            with tc.tile_pool(name="ph1", bufs=1) as ph1:
                h1 = ph1.tile([96, LEAD + BC * H1BLK + LEAD], F32R)
                nc.gpsimd.dma_start(out=h1, in_=AP(
                    tensor=d["zeros"].tensor, offset=0,
                    ap=[[0, 96], [1, LEAD + BC * H1BLK + LEAD]]))
                conv1(d, h1, b1, **ctx_args)
                conv2(d, h1, h2, b2, **ctx_args)
            conv3(d, h2, b3, h3a, h3b, **ctx_args)
        fc_eb(d, out_d, h3a, h3b, ident, fb1, fb2, fb3, gt, noise, dbg,
              **ctx_args)


def conv1(d, h1, b1, nc, tc):
    """R [75, BC*900] @ W1T [75, 96] -> psum [96, 900/sample] -> pool+relu+bias
    -> h1 blocks."""
    CH = 8  # samples per R chunk
    with tc.tile_pool(name="c1w", bufs=1) as wp, \
         tc.tile_pool(name="c1r", bufs=2) as rp, \
         tc.tile_pool(name="c1t", bufs=4) as tp, \
         tc.tile_pool(name="c1ps", bufs=3, space="PSUM") as pp:
        w1 = wp.tile([75, 96], F32R)
        nc.sync.dma_start(out=w1, in_=d["w1t"])
        for c in range(BC // CH):
            r = rp.tile([75, CH * 900], F32R, tag="r")
            nc.sync.dma_start(out=r, in_=d["r1"][:, c * CH * 900:(c + 1) * CH * 900])
            for s in range(CH):
                n = c * CH + s
                ps = pp.tile([96, 1024], F32)
                nc.tensor.matmul(ps[:, 0:512], w1, r[:, s * 900: s * 900 + 512],
                                 start=True, stop=True)
                nc.tensor.matmul(ps[:, 512:900], w1,
                                 r[:, s * 900 + 512: s * 900 + 900],
                                 start=True, stop=True)
                # maxpool 3x3 s2 over 30x30 -> 14x14 (relu after pool)
                sc = tp.tile([96, 900], F32, tag="sc")
                nc.scalar.copy(sc[:, :], ps[:, 0:900])
                px = tp.tile([96, 30 * 14], F32, tag="px")
                nc.vector.tensor_tensor(
                    px[:, :], mkap(sc[:, :], 0, [[30, 30], [2, 14]]),
                    mkap(sc[:, :], 1, [[30, 30], [2, 14]]), AluOpType.max)
                nc.vector.tensor_tensor(
                    px[:, :], px[:, :],
                    mkap(sc[:, :], 2, [[30, 30], [2, 14]]), AluOpType.max)
                po = tp.tile([96, 196], F32, tag="po")
                nc.vector.tensor_tensor(
                    po[:, :], mkap(px[:, :], 0, [[28, 14], [1, 14]]),
                    mkap(px[:, :], 14, [[28, 14], [1, 14]]), AluOpType.max)
                nc.vector.tensor_tensor(
                    po[:, :], po[:, :],
                    mkap(px[:, :], 28, [[28, 14], [1, 14]]), AluOpType.max)
                # relu(pool + b) into h1 block (row/col 1..15)
                dst = mkap(h1[:, :], LEAD + n * H1BLK + H1W + 2,
                           [[H1W, 14], [1, 14]])
                nc.scalar.activation(dst, po[:, :],
                                     mybir.ActivationFunctionType.Relu,
                                     bias=b1[:, :])


def conv_shift(d, wname, src, dst_h, cin, cout, h_in, w_in, chunk, bias,
               out_pool_geom, nc, tc, name):
    """Shared conv2/conv3: shifted-offset matmuls over block-layout src,
    pool+relu+bias into dst blocks (or returned raw tiles for conv3)."""
    raise NotImplementedError


def conv2(d, h1, h2, b2, nc, tc):
    CH = 8                       # samples per psum chunk
    ROWS = CH * H1H              # 128 rows of 16
    with tc.tile_pool(name="c2w", bufs=1) as wp, \
         tc.tile_pool(name="c2t", bufs=4) as tp, \
         tc.tile_pool(name="c2ps", bufs=2, space="PSUM") as pp:
        w2 = wp.tile([96, 25, 128], F32R)
        nc.sync.dma_start(out=w2, in_=d["w2t"])
        for c in range(BC // CH):
            ps = pp.tile([128, CH * H1BLK], F32)  # 2048 cols = 4 banks
            for k, (dy, dx) in enumerate(OFFS):
                for g in range(ROWS // 32):
                    src = mkap(h1[:, :],
                               LEAD + c * CH * H1BLK + (g * 32 + dy) * H1W
                               + 2 + dx,
                               [[H1W, 32], [1, 14]])
                    dst = mkap(ps[:, :], g * 32 * H1W + 2,
                               [[H1W, 32], [1, 14]])
                    nc.tensor.matmul(dst, w2[:, k, :], src,
                                     start=(k == 0), stop=(k == 24),
                                     skip_group_check=True)
            # pool 14x14 -> 6x6 + relu + bias, per sample
            sc = tp.tile([128, CH * H1BLK], F32, tag="sc")
            nc.scalar.copy(sc[:, :], ps[:, :])
            for s in range(CH):
                n = c * CH + s
                base = s * H1BLK + H1W + 2  # row1,col2 of block = out (0,0)
                px = tp.tile([128, 14 * 6], F32, tag="px")
                nc.vector.tensor_tensor(
                    px[:, :], mkap(sc[:, :], base, [[H1W, 14], [2, 6]]),
                    mkap(sc[:, :], base + 1, [[H1W, 14], [2, 6]]), AluOpType.max)
                nc.vector.tensor_tensor(
                    px[:, :], px[:, :],
                    mkap(sc[:, :], base + 2, [[H1W, 14], [2, 6]]), AluOpType.max)
                po = tp.tile([128, 36], F32, tag="po")
                nc.vector.tensor_tensor(
                    po[:, :], mkap(px[:, :], 0, [[12, 6], [1, 6]]),
                    mkap(px[:, :], 6, [[12, 6], [1, 6]]), AluOpType.max)
                nc.vector.tensor_tensor(
                    po[:, :], po[:, :],
                    mkap(px[:, :], 12, [[12, 6], [1, 6]]), AluOpType.max)
                dst = mkap(h2[:, :], LEAD + n * H2BLK + H2W + 2,
                           [[H2W, 6], [1, 6]])
                nc.scalar.activation(dst, po[:, :],
                                     mybir.ActivationFunctionType.Relu,
                                     bias=b2[:, :])


def conv3(d, h2, b3, h3a, h3b, nc, tc):
    CH = 16                      # samples per psum chunk
    ROWS = CH * H2H              # 128 rows of 8
    with tc.tile_pool(name="c3w", bufs=1) as wp, \
         tc.tile_pool(name="c3t", bufs=4) as tp, \
         tc.tile_pool(name="c3ps", bufs=4, space="PSUM") as pp:
        w3 = wp.tile([128, 25, 256], F32R)
        nc.sync.dma_start(out=w3, in_=d["w3t"])
        for c in range(BC // CH):
            for mt, h3 in ((0, h3a), (1, h3b)):
                ps = pp.tile([128, CH * H2BLK], F32)  # 1024 cols = 2 banks
                for k, (dy, dx) in enumerate(OFFS):
                    for g in range(ROWS // 64):
                        src = mkap(h2[:, :],
                                   LEAD + c * CH * H2BLK + (g * 64 + dy) * H2W
                                   + 2 + dx,
                                   [[H2W, 64], [1, 6]])
                        dst = mkap(ps[:, :], g * 64 * H2W + 2,
                                   [[H2W, 64], [1, 6]])
                        nc.tensor.matmul(dst, w3[:, k, 128 * mt:128 * mt + 128],
                                         src, start=(k == 0), stop=(k == 24),
                                         skip_group_check=True)
                sc = tp.tile([128, CH * H2BLK], F32, tag="sc")
                nc.scalar.copy(sc[:, :], ps[:, :])
                for s in range(CH):
                    n = c * CH + s
                    base = s * H2BLK + H2W + 2
                    px = tp.tile([128, 6 * 2], F32, tag="px")
                    nc.vector.tensor_tensor(
                        px[:, :], mkap(sc[:, :], base, [[H2W, 6], [2, 2]]),
                        mkap(sc[:, :], base + 1, [[H2W, 6], [2, 2]]),
                        AluOpType.max)
                    nc.vector.tensor_tensor(
                        px[:, :], px[:, :],
                        mkap(sc[:, :], base + 2, [[H2W, 6], [2, 2]]),
                        AluOpType.max)
                    po = tp.tile([128, 4], F32, tag="po")
                    nc.vector.tensor_tensor(
                        po[:, :], mkap(px[:, :], 0, [[4, 2], [1, 2]]),
                        mkap(px[:, :], 2, [[4, 2], [1, 2]]), AluOpType.max)
                    nc.vector.tensor_tensor(
                        po[:, :], po[:, :],
                        mkap(px[:, :], 4, [[4, 2], [1, 2]]), AluOpType.max)
                    nc.scalar.activation(po[:, :], po[:, :],
                                         mybir.ActivationFunctionType.Relu,
                                         bias=b3[:, mt:mt + 1])
                    nc.vector.tensor_copy(h3[:, n, :], po[:, :])


def fc_eb(d, out_d, h3a, h3b, ident, fb1, fb2, fb3, gt, noise, dbg, nc, tc):
    def dump(nm, ap):
        if dbg:
            nc.sync.dma_start(out=dbg[nm], in_=ap)
    Relu = mybir.ActivationFunctionType.Relu
    Copy = mybir.ActivationFunctionType.Copy
    with tc.tile_pool(name="fca", bufs=1) as fa, \
         tc.tile_pool(name="wstream", bufs=3) as ws, \
         tc.tile_pool(name="wrelu", bufs=2) as wr, \
         tc.tile_pool(name="fct", bufs=1) as ft:

        # ---- h4T [128, 8, 64]: ci = co*4 + s, from h3a/h3b via DMA reshape
        h4t = fa.tile([128, 8, 64], F32R)
        if dbg:
            h4f = fa.tile([128, 8, 64], F32)
        for t in range(8):
            h3 = h3a if t < 4 else h3b
            co0 = 32 * (t % 4)
            for sp in range(4):
                dst = h4t[sp::4, t, :]
                src = h3[co0:co0 + 32, :, sp]
                nc.gpsimd.dma_start(out=dst, in_=src)
        if dbg:
            nc.vector.tensor_copy(h4f[:, :, :], h4t[:, :, :].bitcast(F32))
            nc.sync.dma_start(out=dbg["h4"], in_=h4f[:, :, :])

        # ---- fc1: h5 = relu(h4 @ fc1_w.T + fb1)
        h5 = fa.tile([BC, 2048], F32)
        h5t = fa.tile([128, 16, 64], F32R)
        with tc.tile_pool(name="ps1", bufs=1, space="PSUM") as pp, \
             tc.tile_pool(name="pstr", bufs=4, space="PSUM") as ptr:
            ps = pp.tile([BC, 2048], F32)
            for kt in range(8):
                w = ws.tile([128, 2048], F32R, tag="w")
                nc.sync.dma_start(out=w, in_=d["fc1wt"][128 * kt:128 * kt + 128, :])
                for ch in range(4):
                    nc.tensor.matmul(ps[:, 512 * ch:512 * ch + 512],
                                     h4t[:, kt, :], w[:, 512 * ch:512 * ch + 512],
                                     start=(kt == 0), stop=(kt == 7))
            tmp = ft.tile([BC, 2048], F32, tag="ta")
            nc.vector.tensor_tensor(tmp, ps[:, :], fb1[:, :], AluOpType.add)
            nc.scalar.activation(h5[:, :], tmp[:, :], Relu)
            dump("h5", h5[:, :])
            # h5T via PE transposes
            for kt in range(16):
                pt = ptr.tile([128, 64], F32, tag="tr")
                nc.tensor.transpose(pt[:, :], h5[:, 128 * kt:128 * kt + 128],
                                    ident[:, :])
                nc.scalar.copy(h5t[:, kt, :], pt[:, :])

        # ---- pass1: h6_eb = relu(h5 @ fc2_w.T + fb2); z2 = h5 @ Wp2.T
        h6eb = fa.tile([BC, 2048], F32)
        s2 = fa.tile([BC, 2048], F32)
        with tc.tile_pool(name="ps2", bufs=2, space="PSUM") as pp:
            pa = pp.tile([BC, 2048], F32, tag="big")
            pz = pp.tile([BC, 2048], F32, tag="big")
            for kt in range(16):
                w = ws.tile([128, 2048], F32R, tag="w")
                nc.sync.dma_start(out=w, in_=d["fc2wt"][128 * kt:128 * kt + 128, :])
                wp = wr.tile([128, 2048], F32R, tag="wp")
                nc.scalar.activation(wp[:, :], w[:, :], Relu)
                for ch in range(4):
                    sl = slice(512 * ch, 512 * ch + 512)
                    nc.tensor.matmul(pa[:, sl], h5t[:, kt, :], w[:, sl],
                                     start=(kt == 0), stop=(kt == 15))
                    nc.tensor.matmul(pz[:, sl], h5t[:, kt, :], wp[:, sl],
                                     start=(kt == 0), stop=(kt == 15))
            tmp = ft.tile([BC, 2048], F32, tag="ta")
            nc.vector.tensor_tensor(tmp, pa[:, :], fb2[:, :], AluOpType.add)
            nc.scalar.activation(h6eb[:, :], tmp[:, :], Relu)
            dump("h6eb", h6eb[:, :])
            # EB through fc3: p_h6 = h6eb * g / z3 (z3 = rowsum(h6eb*g))
            hg = ft.tile([BC, 2048], F32, tag="tb")
            nc.vector.tensor_tensor(hg, h6eb[:, :], gt[:, :], AluOpType.mult)
            z3 = ft.tile([BC, 1], F32, tag="z3")
            nc.vector.tensor_reduce(z3, hg[:, :], mybir.AxisListType.X,
                                    AluOpType.add)
            z3c = ft.tile([BC, 1], F32, tag="z3c")
            nc.vector.tensor_scalar_max(z3c, z3[:, :], 1e-30)
            rz3 = ft.tile([BC, 1], F32, tag="rz3")
            nc.vector.reciprocal(rz3, z3c[:, :])
            gt3 = ft.tile([BC, 1], F32, tag="gt3")
            nc.vector.tensor_scalar(gt3, z3[:, :], 0.0, None, AluOpType.is_gt)
            rz3m = ft.tile([BC, 1], F32, tag="rz3m")
            nc.vector.tensor_tensor(rz3m, rz3[:, :], gt3[:, :], AluOpType.mult)
            ph6 = ft.tile([BC, 2048], F32, tag="ta")
            nc.vector.tensor_scalar_mul(ph6, hg[:, :], rz3m[:, :])
            # s2 = where(z2>0, p_h6/z2, 0)
            z2c = ft.tile([BC, 2048], F32, tag="tc")
            nc.vector.tensor_scalar_max(z2c, pz[:, :], 1e-30)
            rz2 = ft.tile([BC, 2048], F32, tag="td")
            nc.vector.reciprocal(rz2, z2c[:, :])
            gt2 = ft.tile([BC, 2048], F32, tag="tc")
            nc.vector.tensor_scalar(gt2, pz[:, :], 0.0, None, AluOpType.is_gt)
            pr = ft.tile([BC, 2048], F32, tag="tb")
            nc.vector.tensor_tensor(pr, ph6[:, :], rz2[:, :], AluOpType.mult)
            nc.vector.tensor_tensor(s2[:, :], pr[:, :], gt2[:, :], AluOpType.mult)
            if dbg:
                zc = ft.tile([BC, 2048], F32, tag="td")
                nc.vector.tensor_copy(zc, pz[:, :])
                dump("z2", zc[:, :])
                dump("s2", s2[:, :])

        # ---- s2T, pass2: r = s2 @ Wp2 ; pebs = h5 * r ; h_ed
        s2t = fa.tile([128, 16, 64], F32R)
        with tc.tile_pool(name="pstr2", bufs=4, space="PSUM") as ptr:
            for kt in range(16):
                pt = ptr.tile([128, 64], F32, tag="tr")
                nc.tensor.transpose(pt[:, :], s2[:, 128 * kt:128 * kt + 128],
                                    ident[:, :])
                nc.scalar.copy(s2t[:, kt, :], pt[:, :])
        h_ed = fa.tile([BC, 2048], F32)
        with tc.tile_pool(name="ps3", bufs=1, space="PSUM") as pp:
            ps = pp.tile([BC, 2048], F32)
            for kt in range(16):
                w = ws.tile([128, 2048], F32R, tag="w")
                nc.sync.dma_start(out=w, in_=d["wp2"][128 * kt:128 * kt + 128, :])
                for ch in range(4):
                    sl = slice(512 * ch, 512 * ch + 512)
                    nc.tensor.matmul(ps[:, sl], s2t[:, kt, :], w[:, sl],
                                     start=(kt == 0), stop=(kt == 15))
            pebs = ft.tile([BC, 2048], F32, tag="ta")
            nc.vector.tensor_tensor(pebs, h5[:, :], ps[:, :], AluOpType.mult)
            # retain_p = (0.5 - 0.5*pebs) / (1023*pebs + 0.5)
            dn = ft.tile([BC, 2048], F32, tag="tb")
            nc.vector.tensor_scalar(dn, pebs[:, :], 1023.0, 0.5,
                                    AluOpType.mult, AluOpType.add)
            rd = ft.tile([BC, 2048], F32, tag="tc")
            nc.vector.reciprocal(rd, dn[:, :])
            t05 = ft.tile([BC, 2048], F32, tag="tb")
            nc.vector.tensor_scalar(t05, pebs[:, :], -0.5, 0.5,
                                    AluOpType.mult, AluOpType.add)
            retain = ft.tile([BC, 2048], F32, tag="td")
            nc.vector.tensor_tensor(retain, t05[:, :], rd[:, :], AluOpType.mult)
            dump("pebs", pebs[:, :])
            dump("retain", retain[:, :])
            mask = ft.tile([BC, 2048], F32, tag="tb")
            nc.vector.tensor_tensor(mask, noise[:, :], retain[:, :],
                                    AluOpType.is_lt)
            rc = ft.tile([BC, 2048], F32, tag="tc")
            nc.vector.tensor_scalar_max(rc, retain[:, :], 1e-30)
            rr = ft.tile([BC, 2048], F32, tag="ta")
            nc.vector.reciprocal(rr, rc[:, :])
            hm = ft.tile([BC, 2048], F32, tag="tc")
            nc.vector.tensor_tensor(hm, h5[:, :], mask[:, :], AluOpType.mult)
            nc.vector.tensor_tensor(h_ed[:, :], hm[:, :], rr[:, :],
                                    AluOpType.mult)
            dump("mask", mask[:, :])
            dump("h_ed", h_ed[:, :])

        # ---- h_edT, pass3: h6 = relu(h_ed @ fc2_w.T + fb2)
        hedt = fa.tile([128, 16, 64], F32R)
        with tc.tile_pool(name="pstr3", bufs=4, space="PSUM") as ptr:
            for kt in range(16):
                pt = ptr.tile([128, 64], F32, tag="tr")
                nc.tensor.transpose(pt[:, :], h_ed[:, 128 * kt:128 * kt + 128],
                                    ident[:, :])
                nc.scalar.copy(hedt[:, kt, :], pt[:, :])
        h6 = fa.tile([BC, 2048], F32)
        with tc.tile_pool(name="ps4", bufs=1, space="PSUM") as pp:
            ps = pp.tile([BC, 2048], F32)
            for kt in range(16):
                w = ws.tile([128, 2048], F32R, tag="w")
                nc.sync.dma_start(out=w, in_=d["fc2wt"][128 * kt:128 * kt + 128, :])
                for ch in range(4):
                    sl = slice(512 * ch, 512 * ch + 512)
                    nc.tensor.matmul(ps[:, sl], hedt[:, kt, :], w[:, sl],
                                     start=(kt == 0), stop=(kt == 15))
            tmp = ft.tile([BC, 2048], F32, tag="ta")
            nc.vector.tensor_tensor(tmp, ps[:, :], fb2[:, :], AluOpType.add)
            nc.scalar.activation(h6[:, :], tmp[:, :], Relu)
            dump("h6", h6[:, :])

        # ---- out = h6 @ fc3_w.T + fb3
        h6t = fa.tile([128, 16, 64], F32R)
        with tc.tile_pool(name="pstr4", bufs=4, space="PSUM") as ptr, \
             tc.tile_pool(name="ps5", bufs=1, space="PSUM") as pp:
            for kt in range(16):
                pt = ptr.tile([128, 64], F32, tag="tr")
                nc.tensor.transpose(pt[:, :], h6[:, 128 * kt:128 * kt + 128],
                                    ident[:, :])
                nc.scalar.copy(h6t[:, kt, :], pt[:, :])
            w3t = fa.tile([128, 16, 10], F32R)
            nc.sync.dma_start(out=w3t, in_=d["fc3wt"])
            po = pp.tile([BC, 10], F32)
            for kt in range(16):
                nc.tensor.matmul(po[:, :], h6t[:, kt, :], w3t[:, kt, :],
                                 start=(kt == 0), stop=(kt == 15))
            ot = ft.tile([BC, 10], F32, tag="ot")
            nc.vector.tensor_tensor(ot, po[:, :], fb3[:, :], AluOpType.add)
            nc.sync.dma_start(out=out_d, in_=ot[:, :])


# ---------------------------------------------------------------- host side

def prep_host(inputs):
    x = np.asarray(inputs["x"], np.float32)            # [512, 3, 32, 32]
    noise = np.asarray(inputs["noise"], np.float32)
    label = np.asarray(inputs["label"]).astype(np.int64)

    # conv1 im2col: R[p=(ky*5+kx)*3+ci, n, y*30+x] = xpad[n, ci, y+ky, x+kx]
    B = x.shape[0]
    xpad = np.zeros((B, 3, 34, 34), np.float32)
    xpad[:, :, 1:33, 1:33] = x
    win = np.lib.stride_tricks.sliding_window_view(
        xpad, (5, 5), axis=(2, 3))                     # [B, 3, 30, 30, 5, 5]
    # -> [ky, kx, ci, n, y, x]
    R = win.transpose(4, 5, 1, 0, 2, 3).reshape(75, B, 900)

    w1t = np.ascontiguousarray(
        np.asarray(inputs["conv1_w"], np.float32)
        .transpose(2, 3, 1, 0).reshape(75, 96))
    w2t = np.ascontiguousarray(
        np.asarray(inputs["conv2_w"], np.float32)
        .transpose(1, 2, 3, 0).reshape(96, 25, 128))
    w3t = np.ascontiguousarray(
        np.asarray(inputs["conv3_w"], np.float32)
        .transpose(1, 2, 3, 0).reshape(128, 25, 256))
    fc1wt = np.ascontiguousarray(np.asarray(inputs["fc1_w"], np.float32).T)
    fc2wt = np.ascontiguousarray(np.asarray(inputs["fc2_w"], np.float32).T)
    wp2 = np.maximum(np.asarray(inputs["fc2_w"], np.float32), 0.0)
    fc3wt = np.ascontiguousarray(
        np.asarray(inputs["fc3_w"], np.float32).T.reshape(16, 128, 10)
        .transpose(1, 0, 2))
    g_all = np.maximum(np.asarray(inputs["fc3_w"], np.float32), 0.0)[label]

    shared = dict(
        w1t=w1t, w2t=w2t, w3t=w3t, fc1wt=fc1wt, fc2wt=fc2wt, wp2=wp2,
        fc3wt=fc3wt,
        b1=np.asarray(inputs["conv1_b"], np.float32),
        b2=np.asarray(inputs["conv2_b"], np.float32),
        b3=np.asarray(inputs["conv3_b"], np.float32),
        fb1=np.asarray(inputs["fc1_b"], np.float32),
        fb2=np.asarray(inputs["fc2_b"], np.float32),
        fb3=np.asarray(inputs["fc3_b"], np.float32),
    )
    in_maps = []
    for c in range(NCORES):
        s = slice(c * BC, (c + 1) * BC)
        m = dict(shared)
        m["r1"] = np.ascontiguousarray(R[:, s, :]).reshape(75, BC * 900)
        m["noise"] = np.ascontiguousarray(noise[s])
        m["g"] = np.ascontiguousarray(g_all[s])
        m["zeros"] = np.zeros(LEAD + BC * H1BLK + LEAD, np.float32)
        in_maps.append(m)
    return in_maps


_CACHED = {}


def kernel(**inputs):
    rep = int(_CACHED.get("rep", 1))
    key = ("nc", rep)
    if key not in _CACHED:
        _CACHED[key] = build_bass(rep)
    nc = _CACHED[key]
    in_maps = prep_host(inputs)
    res = run_bass_kernel_spmd(nc, in_maps, core_ids=list(range(NCORES)))
    out = np.concatenate([r["out"] for r in res.results], axis=0)
    return out.astype(np.float32)


if __name__ == "__main__":
    import reference
    ins = {k: np.asarray(v) for k, v in reference.setup_inputs().items()}
    exp = np.asarray(reference.reference(**ins))
    act = kernel(**ins)
    rel = np.linalg.norm(act - exp) / np.linalg.norm(exp)
    print("Relative error:", rel)


# revision 12
# speedup vs baseline: 2.6529x; 2.6529x over previous
"""TRN2 Bass kernel for nn_CNN_2_EDropout: CNN forward + excitation-backprop
dropout. Data-parallel over 8 NeuronCores (64 samples each). All matmuls in
float32r (full-rate fp32-reduced mode, ~2e-5 rel err).

Layouts (per core, 64 samples):
- conv1 input: host-side im2col R [75=(ky,kx,ci), 64, 900=(30y,30x)]
- conv activations h1/h2 live in SBUF "block" layout [C, n, H, W] where
  H = out_h + 2, W = out_w + 2 with one zero pad row/col on each side; the
  required +-2 conv halo is served by the neighbouring sample's zero pad row,
  so shifted-offset matmuls use a single (rows, x) 3D access pattern whose
  32-row groups never cross PSUM bank boundaries.
- FC/EB stage keeps activations n-major [64, 2048]; contraction-major copies
  (h5T etc.) are built with PE transposes.
"""
import sys
import numpy as np

sys.path.insert(0, '/opt/trn_rl_repo')

import concourse.bass as bass  # noqa: E402
import concourse.tile as tile  # noqa: E402
from concourse import bacc, mybir  # noqa: E402
from concourse.bass import AP  # noqa: E402
from concourse.bass_utils import run_bass_kernel_spmd  # noqa: E402
from concourse.masks import make_identity  # noqa: E402
from concourse.alu_op_type import AluOpType  # noqa: E402

F32 = mybir.dt.float32
F32R = mybir.dt.float32r

NCORES = 8
BC = 64          # samples per core
P_KEEP = 0.5

# conv1: 3ch 32x32, k5 pad1 -> 30x30, pool -> 14x14
# conv2: 96 -> 128, k5 pad2 on 14x14 -> 14x14, pool -> 6x6
# conv3: 128 -> 256, k5 pad2 on 6x6 -> 6x6, pool -> 2x2
H1H, H1W = 16, 16      # h1 block: 1 zero pad row/col each side of 14x14
H1BLK = H1H * H1W      # 256
H2H, H2W = 8, 8        # h2 block: 1 pad around 6x6
H2BLK = H2H * H2W      # 64
LEAD = 32              # zeroed margin elems before/after block arrays

OFFS = [(dy, dx) for dy in range(-2, 3) for dx in range(-2, 3)]


def _xrange(dx, w_out):
    """Valid out-x range [x0, x0+cnt) for conv offset dx with 1-col padding."""
    x0 = 1 if dx == -2 else 0
    x1 = w_out - 1 if dx == 2 else w_out
    return x0, x1 - x0


def mkap(base: AP, off: int, dims):
    """AP at base.offset+off with free dims `dims` ([[step, count], ...]),
    keeping the partition dim of `base`."""
    return AP(tensor=base.tensor, offset=base.offset + off,
              ap=[list(base.ap[0])] + [list(d) for d in dims])


def build_bass(rep: int = 1, part: str = "all"):
    nc = bacc.Bacc("TRN2", target_bir_lowering=False, debug=False,
                   num_devices=NCORES)
    d = {}

    def din(name, shape, dt=F32R):
        d[name] = nc.dram_tensor(name, list(shape), dt, kind="ExternalInput").ap()

    din("r1", [75, BC * 900])
    din("w1t", [75, 96])
    din("w2t", [96, 25, 128])
    din("w3t", [128, 25, 256])
    din("fc1wt", [1024, 2048])
    din("fc2wt", [2048, 2048])
    din("wp2", [2048, 2048])
    din("fc3wt", [128, 16, 10])
    din("b1", [96], F32)
    din("b2", [128], F32)
    din("b3", [256], F32)
    din("fb1", [2048], F32)
    din("fb2", [2048], F32)
    din("fb3", [10], F32)
    din("g", [BC, 2048], F32)
    din("noise", [BC, 2048], F32)
    din("zeros", [LEAD + BC * H1BLK + LEAD])
    out_d = nc.dram_tensor("out", [BC, 10], F32, kind="ExternalOutput").ap()
    import os
    dbg = {}
    if os.environ.get("DBG_DUMP", "0") == "1":
        for nm in ("h5", "h6eb", "z2", "s2", "pebs", "retain", "mask", "h_ed",
                   "h6"):
            dbg[nm] = nc.dram_tensor("dbg_" + nm, [BC, 2048], F32,
                                     kind="ExternalOutput").ap()
        dbg["h4"] = nc.dram_tensor("dbg_h4", [128, 8, 64], F32,
                                   kind="ExternalOutput").ap()

    tc = tile.TileContext(nc, num_cores=NCORES)
    with tc:
        with tc.tile_pool(name="perm", bufs=1) as perm:
            # persistent across reps: biases, identity
            ident = perm.tile([64, 64], F32)
            make_identity(nc, ident)
            b1 = perm.tile([96, 1], F32)
            nc.sync.dma_start(out=b1, in_=d["b1"].rearrange("(c o) -> c o", o=1))
            b2 = perm.tile([128, 1], F32)
            nc.sync.dma_start(out=b2, in_=d["b2"].rearrange("(c o) -> c o", o=1))
            b3 = perm.tile([256 // 2, 2], F32)  # [co%128, tile] per M-tile
            nc.sync.dma_start(
                out=b3, in_=d["b3"].rearrange("(t c) -> c t", t=2))
            # fc bias rows replicated to 64 partitions
            fb1 = perm.tile([BC, 2048], F32)
            nc.gpsimd.dma_start(out=fb1, in_=AP(
                tensor=d["fb1"].tensor, offset=0, ap=[[0, BC], [1, 2048]]))
            fb2 = perm.tile([BC, 2048], F32)
            nc.gpsimd.dma_start(out=fb2, in_=AP(
                tensor=d["fb2"].tensor, offset=0, ap=[[0, BC], [1, 2048]]))
            fb3 = perm.tile([BC, 10], F32)
            nc.gpsimd.dma_start(out=fb3, in_=AP(
                tensor=d["fb3"].tensor, offset=0, ap=[[0, BC], [1, 10]]))
            gt = perm.tile([BC, 2048], F32)
            nc.sync.dma_start(out=gt, in_=d["g"])
            noise = perm.tile([BC, 2048], F32)
            nc.sync.dma_start(out=noise, in_=d["noise"])

            for _ in range(rep):
                one_pass(nc, tc, d, out_d, ident, b1, b2, b3, fb1, fb2, fb3,
                         gt, noise, dbg, part)
    nc.finalize()
    return nc


def one_pass(nc, tc, d, out_d, ident, b1, b2, b3, fb1, fb2, fb3, gt, noise,
             dbg, part="all"):
    ctx_args = dict(nc=nc, tc=tc)

    with tc.tile_pool(name="acts", bufs=1) as acts:
        h3a = acts.tile([128, BC, 4], F32)
        h3b = acts.tile([128, BC, 4], F32)
        if part == "fc":
            nc.gpsimd.dma_start(out=h3a, in_=AP(
                tensor=d["zeros"].tensor, offset=0, ap=[[0, 128], [1, BC * 4]]))
            nc.gpsimd.dma_start(out=h3b, in_=AP(
                tensor=d["zeros"].tensor, offset=0, ap=[[0, 128], [1, BC * 4]]))
            fc_eb(d, out_d, h3a, h3b, ident, fb1, fb2, fb3, gt, noise, dbg,
                  **ctx_args)
            return
        with tc.tile_pool(name="ph2", bufs=1) as ph2:
            h2 = ph2.tile([128, LEAD + BC * H2BLK + LEAD], F32R)
            nc.gpsimd.dma_start(out=h2, in_=AP(
                tensor=d["zeros"].tensor, offset=0,
                ap=[[0, 128], [1, LEAD + BC * H2BLK + LEAD]]))
            with tc.tile_pool(name="ph1", bufs=1) as ph1:
                h1 = ph1.tile([96, LEAD + BC * H1BLK + LEAD], F32R)
                nc.gpsimd.dma_start(out=h1, in_=AP(
                    tensor=d["zeros"].tensor, offset=0,
                    ap=[[0, 96], [1, LEAD + BC * H1BLK + LEAD]]))
                conv1(d, h1, b1, **ctx_args)
                conv2(d, h1, h2, b2, **ctx_args)
            conv3(d, h2, b3, h3a, h3b, **ctx_args)
        if part == "conv":
            ot = acts.tile([BC, 10], F32, tag="oc")
            nc.vector.tensor_copy(ot[:, :4], h3a[0:BC, 0, :])
            nc.sync.dma_start(out=out_d, in_=ot[:, :])
            return
        fc_eb(d, out_d, h3a, h3b, ident, fb1, fb2, fb3, gt, noise, dbg,
              **ctx_args)


def conv1(d, h1, b1, nc, tc):
    """R [75, BC*900] @ W1T [75, 96] -> psum [96, 900/sample] -> pool+relu+bias
    -> h1 blocks."""
    CH = 8  # samples per R chunk
    with tc.tile_pool(name="c1w", bufs=1) as wp, \
         tc.tile_pool(name="c1r", bufs=2) as rp, \
         tc.tile_pool(name="c1t", bufs=4) as tp, \
         tc.tile_pool(name="c1ps", bufs=3, space="PSUM") as pp:
        w1 = wp.tile([75, 96], F32R)
        nc.sync.dma_start(out=w1, in_=d["w1t"])
        for c in range(BC // CH):
            r = rp.tile([75, CH * 900], F32R, tag="r")
            nc.sync.dma_start(out=r, in_=d["r1"][:, c * CH * 900:(c + 1) * CH * 900])
            for s in range(CH):
                n = c * CH + s
                ps = pp.tile([96, 1024], F32)
                nc.tensor.matmul(ps[:, 0:512], w1, r[:, s * 900: s * 900 + 512],
                                 start=True, stop=True)
                nc.tensor.matmul(ps[:, 512:900], w1,
                                 r[:, s * 900 + 512: s * 900 + 900],
                                 start=True, stop=True)
                # maxpool 3x3 s2 over 30x30 -> 14x14 (relu after pool)
                sc = tp.tile([96, 900], F32, tag="sc")
                nc.scalar.copy(sc[:, :], ps[:, 0:900])
                px = tp.tile([96, 30 * 14], F32, tag="px")
                nc.vector.tensor_tensor(
                    px[:, :], mkap(sc[:, :], 0, [[30, 30], [2, 14]]),
                    mkap(sc[:, :], 1, [[30, 30], [2, 14]]), AluOpType.max)
                nc.vector.tensor_tensor(
                    px[:, :], px[:, :],
                    mkap(sc[:, :], 2, [[30, 30], [2, 14]]), AluOpType.max)
                po = tp.tile([96, 196], F32, tag="po")
                nc.vector.tensor_tensor(
                    po[:, :], mkap(px[:, :], 0, [[28, 14], [1, 14]]),
                    mkap(px[:, :], 14, [[28, 14], [1, 14]]), AluOpType.max)
                nc.vector.tensor_tensor(
                    po[:, :], po[:, :],
                    mkap(px[:, :], 28, [[28, 14], [1, 14]]), AluOpType.max)
                # relu(pool + b) into h1 block (row/col 1..15)
                dst = mkap(h1[:, :], LEAD + n * H1BLK + H1W + 2,
                           [[H1W, 14], [1, 14]])
                nc.scalar.activation(dst, po[:, :],
                                     mybir.ActivationFunctionType.Relu,
                                     bias=b1[:, :])


def conv_shift(d, wname, src, dst_h, cin, cout, h_in, w_in, chunk, bias,
               out_pool_geom, nc, tc, name):
    """Shared conv2/conv3: shifted-offset matmuls over block-layout src,
    pool+relu+bias into dst blocks (or returned raw tiles for conv3)."""
    raise NotImplementedError


def conv2(d, h1, h2, b2, nc, tc):
    CH = 8                       # samples per psum chunk
    ROWS = CH * H1H              # 128 rows of 16
    with tc.tile_pool(name="c2w", bufs=1) as wp, \
         tc.tile_pool(name="c2t", bufs=4) as tp, \
         tc.tile_pool(name="c2ps", bufs=2, space="PSUM") as pp:
        w2 = wp.tile([96, 25, 128], F32R)
        nc.sync.dma_start(out=w2, in_=d["w2t"])
        for c in range(BC // CH):
            ps = pp.tile([128, CH * H1BLK], F32)  # 2048 cols = 4 banks
            for k, (dy, dx) in enumerate(OFFS):
                for g in range(ROWS // 32):
                    src = mkap(h1[:, :],
                               LEAD + c * CH * H1BLK + (g * 32 + dy) * H1W
                               + 2 + dx,
                               [[H1W, 32], [1, 14]])
                    dst = mkap(ps[:, :], g * 32 * H1W + 2,
                               [[H1W, 32], [1, 14]])
                    nc.tensor.matmul(dst, w2[:, k, :], src,
                                     start=(k == 0), stop=(k == 24),
                                     skip_group_check=True)
            # pool 14x14 -> 6x6 + relu + bias, per sample
            sc = tp.tile([128, CH * H1BLK], F32, tag="sc")
            nc.scalar.copy(sc[:, :], ps[:, :])
            for s in range(CH):
                n = c * CH + s
                base = s * H1BLK + H1W + 2  # row1,col2 of block = out (0,0)
                px = tp.tile([128, 14 * 6], F32, tag="px")
                nc.vector.tensor_tensor(
                    px[:, :], mkap(sc[:, :], base, [[H1W, 14], [2, 6]]),
                    mkap(sc[:, :], base + 1, [[H1W, 14], [2, 6]]), AluOpType.max)
                nc.vector.tensor_tensor(
                    px[:, :], px[:, :],
                    mkap(sc[:, :], base + 2, [[H1W, 14], [2, 6]]), AluOpType.max)
                po = tp.tile([128, 36], F32, tag="po")
                nc.vector.tensor_tensor(
                    po[:, :], mkap(px[:, :], 0, [[12, 6], [1, 6]]),
                    mkap(px[:, :], 6, [[12, 6], [1, 6]]), AluOpType.max)
                nc.vector.tensor_tensor(
                    po[:, :], po[:, :],
                    mkap(px[:, :], 12, [[12, 6], [1, 6]]), AluOpType.max)
                dst = mkap(h2[:, :], LEAD + n * H2BLK + H2W + 2,
                           [[H2W, 6], [1, 6]])
                nc.scalar.activation(dst, po[:, :],
                                     mybir.ActivationFunctionType.Relu,
                                     bias=b2[:, :])


def conv3(d, h2, b3, h3a, h3b, nc, tc):
    CH = 16                      # samples per psum chunk
    ROWS = CH * H2H              # 128 rows of 8
    with tc.tile_pool(name="c3w", bufs=1) as wp, \
         tc.tile_pool(name="c3t", bufs=4) as tp, \
         tc.tile_pool(name="c3ps", bufs=4, space="PSUM") as pp:
        w3 = wp.tile([128, 25, 256], F32R)
        nc.sync.dma_start(out=w3, in_=d["w3t"])
        for c in range(BC // CH):
            for mt, h3 in ((0, h3a), (1, h3b)):
                ps = pp.tile([128, CH * H2BLK], F32)  # 1024 cols = 2 banks
                for k, (dy, dx) in enumerate(OFFS):
                    for g in range(ROWS // 64):
                        src = mkap(h2[:, :],
                                   LEAD + c * CH * H2BLK + (g * 64 + dy) * H2W
                                   + 2 + dx,
                                   [[H2W, 64], [1, 6]])
                        dst = mkap(ps[:, :], g * 64 * H2W + 2,
                                   [[H2W, 64], [1, 6]])
                        nc.tensor.matmul(dst, w3[:, k, 128 * mt:128 * mt + 128],
                                         src, start=(k == 0), stop=(k == 24),
                                         skip_group_check=True)
                sc = tp.tile([128, CH * H2BLK], F32, tag="sc")
                nc.scalar.copy(sc[:, :], ps[:, :])
                for s in range(CH):
                    n = c * CH + s
                    base = s * H2BLK + H2W + 2
                    px = tp.tile([128, 6 * 2], F32, tag="px")
                    nc.vector.tensor_tensor(
                        px[:, :], mkap(sc[:, :], base, [[H2W, 6], [2, 2]]),
                        mkap(sc[:, :], base + 1, [[H2W, 6], [2, 2]]),
                        AluOpType.max)
                    nc.vector.tensor_tensor(
                        px[:, :], px[:, :],
                        mkap(sc[:, :], base + 2, [[H2W, 6], [2, 2]]),
                        AluOpType.max)
                    po = tp.tile([128, 4], F32, tag="po")
                    nc.vector.tensor_tensor(
                        po[:, :], mkap(px[:, :], 0, [[4, 2], [1, 2]]),
                        mkap(px[:, :], 2, [[4, 2], [1, 2]]), AluOpType.max)
                    nc.vector.tensor_tensor(
                        po[:, :], po[:, :],
                        mkap(px[:, :], 4, [[4, 2], [1, 2]]), AluOpType.max)
                    nc.scalar.activation(po[:, :], po[:, :],
                                         mybir.ActivationFunctionType.Relu,
                                         bias=b3[:, mt:mt + 1])
                    nc.vector.tensor_copy(h3[:, n, :], po[:, :])


def fc_eb(d, out_d, h3a, h3b, ident, fb1, fb2, fb3, gt, noise, dbg, nc, tc):
    def dump(nm, ap):
        if dbg:
            nc.sync.dma_start(out=dbg[nm], in_=ap)
    Relu = mybir.ActivationFunctionType.Relu
    Copy = mybir.ActivationFunctionType.Copy
    with tc.tile_pool(name="fca", bufs=1) as fa, \
         tc.tile_pool(name="wstream", bufs=3) as ws, \
         tc.tile_pool(name="wrelu", bufs=2) as wr, \
         tc.tile_pool(name="fct", bufs=1) as ft:

        # ---- h4T [128, 8, 64]: ci = co*4 + s, from h3a/h3b via DMA reshape
        h4t = fa.tile([128, 8, 64], F32R)
        if dbg:
            h4f = fa.tile([128, 8, 64], F32)
        for t in range(8):
            h3 = h3a if t < 4 else h3b
            co0 = 32 * (t % 4)
            for sp in range(4):
                dst = h4t[sp::4, t, :]
                src = h3[co0:co0 + 32, :, sp]
                nc.gpsimd.dma_start(out=dst, in_=src)
        if dbg:
            nc.vector.tensor_copy(h4f[:, :, :], h4t[:, :, :].bitcast(F32))
            nc.sync.dma_start(out=dbg["h4"], in_=h4f[:, :, :])

        # ---- fc1: h5 = relu(h4 @ fc1_w.T + fb1)
        h5 = fa.tile([BC, 2048], F32)
        h5t = fa.tile([128, 16, 64], F32R)
        with tc.tile_pool(name="ps1", bufs=1, space="PSUM") as pp, \
             tc.tile_pool(name="pstr", bufs=4, space="PSUM") as ptr:
            ps = pp.tile([BC, 2048], F32)
            for kt in range(8):
                w = ws.tile([128, 2048], F32R, tag="w")
                nc.sync.dma_start(out=w, in_=d["fc1wt"][128 * kt:128 * kt + 128, :])
                for ch in range(4):
                    nc.tensor.matmul(ps[:, 512 * ch:512 * ch + 512],
                                     h4t[:, kt, :], w[:, 512 * ch:512 * ch + 512],
                                     start=(kt == 0), stop=(kt == 7))
            tmp = ft.tile([BC, 2048], F32, tag="ta")
            nc.vector.tensor_tensor(tmp, ps[:, :], fb1[:, :], AluOpType.add)
            nc.scalar.activation(h5[:, :], tmp[:, :], Relu)
            dump("h5", h5[:, :])
            # h5T via PE transposes
            for kt in range(16):
                pt = ptr.tile([128, 64], F32, tag="tr")
                nc.tensor.transpose(pt[:, :], h5[:, 128 * kt:128 * kt + 128],
                                    ident[:, :])
                nc.scalar.copy(h5t[:, kt, :], pt[:, :])

        # ---- pass1: h6_eb = relu(h5 @ fc2_w.T + fb2); z2 = h5 @ Wp2.T
        h6eb = fa.tile([BC, 2048], F32)
        s2 = fa.tile([BC, 2048], F32)
        with tc.tile_pool(name="ps2", bufs=2, space="PSUM") as pp:
            pa = pp.tile([BC, 2048], F32, tag="big")
            pz = pp.tile([BC, 2048], F32, tag="big")
            for kt in range(16):
                w = ws.tile([128, 2048], F32R, tag="w")
                nc.sync.dma_start(out=w, in_=d["fc2wt"][128 * kt:128 * kt + 128, :])
                wp = wr.tile([128, 2048], F32R, tag="wp")
                nc.scalar.activation(wp[:, :], w[:, :], Relu)
                for ch in range(4):
                    sl = slice(512 * ch, 512 * ch + 512)
                    nc.tensor.matmul(pa[:, sl], h5t[:, kt, :], w[:, sl],
                                     start=(kt == 0), stop=(kt == 15))
                    nc.tensor.matmul(pz[:, sl], h5t[:, kt, :], wp[:, sl],
                                     start=(kt == 0), stop=(kt == 15))
            tmp = ft.tile([BC, 2048], F32, tag="ta")
            nc.vector.tensor_tensor(tmp, pa[:, :], fb2[:, :], AluOpType.add)
            nc.scalar.activation(h6eb[:, :], tmp[:, :], Relu)
            dump("h6eb", h6eb[:, :])
            # EB through fc3: p_h6 = h6eb * g / z3 (z3 = rowsum(h6eb*g))
            hg = ft.tile([BC, 2048], F32, tag="tb")
            nc.vector.tensor_tensor(hg, h6eb[:, :], gt[:, :], AluOpType.mult)
            z3 = ft.tile([BC, 1], F32, tag="z3")
            nc.vector.tensor_reduce(z3, hg[:, :], mybir.AxisListType.X,
                                    AluOpType.add)
            z3c = ft.tile([BC, 1], F32, tag="z3c")
            nc.vector.tensor_scalar_max(z3c, z3[:, :], 1e-30)
            rz3 = ft.tile([BC, 1], F32, tag="rz3")
            nc.vector.reciprocal(rz3, z3c[:, :])
            gt3 = ft.tile([BC, 1], F32, tag="gt3")
            nc.vector.tensor_scalar(gt3, z3[:, :], 0.0, None, AluOpType.is_gt)
            rz3m = ft.tile([BC, 1], F32, tag="rz3m")
            nc.vector.tensor_tensor(rz3m, rz3[:, :], gt3[:, :], AluOpType.mult)
            ph6 = ft.tile([BC, 2048], F32, tag="ta")
            nc.vector.tensor_scalar_mul(ph6, hg[:, :], rz3m[:, :])
            # s2 = where(z2>0, p_h6/z2, 0)
            z2c = ft.tile([BC, 2048], F32, tag="tc")
            nc.vector.tensor_scalar_max(z2c, pz[:, :], 1e-30)
            rz2 = ft.tile([BC, 2048], F32, tag="td")
            nc.vector.reciprocal(rz2, z2c[:, :])
            gt2 = ft.tile([BC, 2048], F32, tag="tc")
            nc.vector.tensor_scalar(gt2, pz[:, :], 0.0, None, AluOpType.is_gt)
            pr = ft.tile([BC, 2048], F32, tag="tb")
            nc.vector.tensor_tensor(pr, ph6[:, :], rz2[:, :], AluOpType.mult)
            nc.vector.tensor_tensor(s2[:, :], pr[:, :], gt2[:, :], AluOpType.mult)
            if dbg:
                zc = ft.tile([BC, 2048], F32, tag="td")
                nc.vector.tensor_copy(zc, pz[:, :])
                dump("z2", zc[:, :])
                dump("s2", s2[:, :])

        # ---- s2T, pass2: r = s2 @ Wp2 ; pebs = h5 * r ; h_ed
        s2t = fa.tile([128, 16, 64], F32R)
        with tc.tile_pool(name="pstr2", bufs=4, space="PSUM") as ptr:
            for kt in range(16):
                pt = ptr.tile([128, 64], F32, tag="tr")
                nc.tensor.transpose(pt[:, :], s2[:, 128 * kt:128 * kt + 128],
                                    ident[:, :])
                nc.scalar.copy(s2t[:, kt, :], pt[:, :])
        h_ed = fa.tile([BC, 2048], F32)
        with tc.tile_pool(name="ps3", bufs=1, space="PSUM") as pp:
            ps = pp.tile([BC, 2048], F32)
            for kt in range(16):
                w = ws.tile([128, 2048], F32R, tag="w")
                nc.sync.dma_start(out=w, in_=d["wp2"][128 * kt:128 * kt + 128, :])
                for ch in range(4):
                    sl = slice(512 * ch, 512 * ch + 512)
                    nc.tensor.matmul(ps[:, sl], s2t[:, kt, :], w[:, sl],
                                     start=(kt == 0), stop=(kt == 15))
            pebs = ft.tile([BC, 2048], F32, tag="ta")
            nc.vector.tensor_tensor(pebs, h5[:, :], ps[:, :], AluOpType.mult)
            # retain_p = (0.5 - 0.5*pebs) / (1023*pebs + 0.5)
            dn = ft.tile([BC, 2048], F32, tag="tb")
            nc.vector.tensor_scalar(dn, pebs[:, :], 1023.0, 0.5,
                                    AluOpType.mult, AluOpType.add)
            rd = ft.tile([BC, 2048], F32, tag="tc")
            nc.vector.reciprocal(rd, dn[:, :])
            t05 = ft.tile([BC, 2048], F32, tag="tb")
            nc.vector.tensor_scalar(t05, pebs[:, :], -0.5, 0.5,
                                    AluOpType.mult, AluOpType.add)
            retain = ft.tile([BC, 2048], F32, tag="td")
            nc.vector.tensor_tensor(retain, t05[:, :], rd[:, :], AluOpType.mult)
            dump("pebs", pebs[:, :])
            dump("retain", retain[:, :])
            mask = ft.tile([BC, 2048], F32, tag="tb")
            nc.vector.tensor_tensor(mask, noise[:, :], retain[:, :],
                                    AluOpType.is_lt)
            rc = ft.tile([BC, 2048], F32, tag="tc")
            nc.vector.tensor_scalar_max(rc, retain[:, :], 1e-30)
            rr = ft.tile([BC, 2048], F32, tag="ta")
            nc.vector.reciprocal(rr, rc[:, :])
            hm = ft.tile([BC, 2048], F32, tag="tc")
            nc.vector.tensor_tensor(hm, h5[:, :], mask[:, :], AluOpType.mult)
            nc.vector.tensor_tensor(h_ed[:, :], hm[:, :], rr[:, :],
                                    AluOpType.mult)
            dump("mask", mask[:, :])
            dump("h_ed", h_ed[:, :])

        # ---- h_edT, pass3: h6 = relu(h_ed @ fc2_w.T + fb2)
        hedt = fa.tile([128, 16, 64], F32R)
        with tc.tile_pool(name="pstr3", bufs=4, space="PSUM") as ptr:
            for kt in range(16):
                pt = ptr.tile([128, 64], F32, tag="tr")
                nc.tensor.transpose(pt[:, :], h_ed[:, 128 * kt:128 * kt + 128],
                                    ident[:, :])
                nc.scalar.copy(hedt[:, kt, :], pt[:, :])
        h6 = fa.tile([BC, 2048], F32)
        with tc.tile_pool(name="ps4", bufs=1, space="PSUM") as pp:
            ps = pp.tile([BC, 2048], F32)
            for kt in range(16):
                w = ws.tile([128, 2048], F32R, tag="w")
                nc.sync.dma_start(out=w, in_=d["fc2wt"][128 * kt:128 * kt + 128, :])
                for ch in range(4):
                    sl = slice(512 * ch, 512 * ch + 512)
                    nc.tensor.matmul(ps[:, sl], hedt[:, kt, :], w[:, sl],
                                     start=(kt == 0), stop=(kt == 15))
            tmp = ft.tile([BC, 2048], F32, tag="ta")
            nc.vector.tensor_tensor(tmp, ps[:, :], fb2[:, :], AluOpType.add)
            nc.scalar.activation(h6[:, :], tmp[:, :], Relu)
            dump("h6", h6[:, :])

        # ---- out = h6 @ fc3_w.T + fb3
        h6t = fa.tile([128, 16, 64], F32R)
        with tc.tile_pool(name="pstr4", bufs=4, space="PSUM") as ptr, \
             tc.tile_pool(name="ps5", bufs=1, space="PSUM") as pp:
            for kt in range(16):
                pt = ptr.tile([128, 64], F32, tag="tr")
                nc.tensor.transpose(pt[:, :], h6[:, 128 * kt:128 * kt + 128],
                                    ident[:, :])
                nc.scalar.copy(h6t[:, kt, :], pt[:, :])
            w3t = fa.tile([128, 16, 10], F32R)
            nc.sync.dma_start(out=w3t, in_=d["fc3wt"])
            po = pp.tile([BC, 10], F32)
            for kt in range(16):
                nc.tensor.matmul(po[:, :], h6t[:, kt, :], w3t[:, kt, :],
                                 start=(kt == 0), stop=(kt == 15))
            ot = ft.tile([BC, 10], F32, tag="ot")
            nc.vector.tensor_tensor(ot, po[:, :], fb3[:, :], AluOpType.add)
            nc.sync.dma_start(out=out_d, in_=ot[:, :])


# ---------------------------------------------------------------- host side

def prep_host(inputs):
    x = np.asarray(inputs["x"], np.float32)            # [512, 3, 32, 32]
    noise = np.asarray(inputs["noise"], np.float32)
    label = np.asarray(inputs["label"]).astype(np.int64)

    # conv1 im2col: R[p=(ky*5+kx)*3+ci, n, y*30+x] = xpad[n, ci, y+ky, x+kx]
    B = x.shape[0]
    xpad = np.zeros((B, 3, 34, 34), np.float32)
    xpad[:, :, 1:33, 1:33] = x
    win = np.lib.stride_tricks.sliding_window_view(
        xpad, (5, 5), axis=(2, 3))                     # [B, 3, 30, 30, 5, 5]
    # -> [ky, kx, ci, n, y, x]
    R = win.transpose(4, 5, 1, 0, 2, 3).reshape(75, B, 900)

    w1t = np.ascontiguousarray(
        np.asarray(inputs["conv1_w"], np.float32)
        .transpose(2, 3, 1, 0).reshape(75, 96))
    w2t = np.ascontiguousarray(
        np.asarray(inputs["conv2_w"], np.float32)
        .transpose(1, 2, 3, 0).reshape(96, 25, 128))
    w3t = np.ascontiguousarray(
        np.asarray(inputs["conv3_w"], np.float32)
        .transpose(1, 2, 3, 0).reshape(128, 25, 256))
    fc1wt = np.ascontiguousarray(np.asarray(inputs["fc1_w"], np.float32).T)
    fc2wt = np.ascontiguousarray(np.asarray(inputs["fc2_w"], np.float32).T)
    wp2 = np.maximum(np.asarray(inputs["fc2_w"], np.float32), 0.0)
    fc3wt = np.ascontiguousarray(
        np.asarray(inputs["fc3_w"], np.float32).T.reshape(16, 128, 10)
        .transpose(1, 0, 2))
    g_all = np.maximum(np.asarray(inputs["fc3_w"], np.float32), 0.0)[label]

    shared = dict(
        w1t=w1t, w2t=w2t, w3t=w3t, fc1wt=fc1wt, fc2wt=fc2wt, wp2=wp2,
        fc3wt=fc3wt,
        b1=np.asarray(inputs["conv1_b"], np.float32),
        b2=np.asarray(inputs["conv2_b"], np.float32),
        b3=np.asarray(inputs["conv3_b"], np.float32),
        fb1=np.asarray(inputs["fc1_b"], np.float32),
        fb2=np.asarray(inputs["fc2_b"], np.float32),
        fb3=np.asarray(inputs["fc3_b"], np.float32),
    )
    in_maps = []
    for c in range(NCORES):
        s = slice(c * BC, (c + 1) * BC)
        m = dict(shared)
        m["r1"] = np.ascontiguousarray(R[:, s, :]).reshape(75, BC * 900)
        m["noise"] = np.ascontiguousarray(noise[s])
        m["g"] = np.ascontiguousarray(g_all[s])
        m["zeros"] = np.zeros(LEAD + BC * H1BLK + LEAD, np.float32)
        in_maps.append(m)
    return in_maps


_CACHED = {}


def kernel(**inputs):
    rep = int(_CACHED.get("rep", 1))
    key = ("nc", rep)
    if key not in _CACHED:
        _CACHED[key] = build_bass(rep)
    nc = _CACHED[key]
    in_maps = prep_host(inputs)
    res = run_bass_kernel_spmd(nc, in_maps, core_ids=list(range(NCORES)))
    out = np.concatenate([r["out"] for r in res.results], axis=0)
    return out.astype(np.float32)


if __name__ == "__main__":
    import reference
    ins = {k: np.asarray(v) for k, v in reference.setup_inputs().items()}
    exp = np.asarray(reference.reference(**ins))
    act = kernel(**ins)
    rel = np.linalg.norm(act - exp) / np.linalg.norm(exp)
    print("Relative error:", rel)


# revision 13
# speedup vs baseline: 3.8132x; 1.4374x over previous
"""TRN2 Bass kernel for nn_CNN_2_EDropout: CNN forward + excitation-backprop
dropout. Data-parallel over 8 NeuronCores (64 samples each). All matmuls in
float32r (full-rate fp32-reduced mode, ~2e-5 rel err).

Layouts (per core, 64 samples):
- conv1 input: host-side im2col R [75=(ky,kx,ci), 64, 900=(30y,30x)]
- conv activations h1/h2 live in SBUF "block" layout [C, n, H, W] where
  H = out_h + 2, W = out_w + 2 with one zero pad row/col on each side; the
  required +-2 conv halo is served by the neighbouring sample's zero pad row,
  so shifted-offset matmuls use a single (rows, x) 3D access pattern whose
  32-row groups never cross PSUM bank boundaries.
- FC/EB stage keeps activations n-major [64, 2048]; contraction-major copies
  (h5T etc.) are built with PE transposes.
"""
import sys
import numpy as np

sys.path.insert(0, '/opt/trn_rl_repo')

import concourse.bass as bass  # noqa: E402
import concourse.tile as tile  # noqa: E402
from concourse import bacc, mybir  # noqa: E402
from concourse.bass import AP  # noqa: E402
from concourse.bass_utils import run_bass_kernel_spmd  # noqa: E402
from concourse.masks import make_identity  # noqa: E402
from concourse.alu_op_type import AluOpType  # noqa: E402

F32 = mybir.dt.float32
F32R = mybir.dt.float32r

NCORES = 8
BC = 64          # samples per core
P_KEEP = 0.5

# conv1: 3ch 32x32, k5 pad1 -> 30x30, pool -> 14x14
# conv2: 96 -> 128, k5 pad2 on 14x14 -> 14x14, pool -> 6x6
# conv3: 128 -> 256, k5 pad2 on 6x6 -> 6x6, pool -> 2x2
H1H, H1W = 16, 16      # h1 block: 1 zero pad row/col each side of 14x14
H1BLK = H1H * H1W      # 256
H2H, H2W = 8, 8        # h2 block: 1 pad around 6x6
H2BLK = H2H * H2W      # 64
LEAD = 32              # zeroed margin elems before/after block arrays

OFFS = [(dy, dx) for dy in range(-2, 3) for dx in range(-2, 3)]


def _xrange(dx, w_out):
    """Valid out-x range [x0, x0+cnt) for conv offset dx with 1-col padding."""
    x0 = 1 if dx == -2 else 0
    x1 = w_out - 1 if dx == 2 else w_out
    return x0, x1 - x0


def mkap(base: AP, off: int, dims):
    """AP at base.offset+off with free dims `dims` ([[step, count], ...]),
    keeping the partition dim of `base`."""
    return AP(tensor=base.tensor, offset=base.offset + off,
              ap=[list(base.ap[0])] + [list(d) for d in dims])


def build_bass(rep: int = 1, part: str = "all"):
    nc = bacc.Bacc("TRN2", target_bir_lowering=False, debug=False,
                   num_devices=NCORES)
    d = {}

    def din(name, shape, dt=F32R):
        d[name] = nc.dram_tensor(name, list(shape), dt, kind="ExternalInput").ap()

    din("r1", [75, BC * 900])
    din("w1t", [75, 96])
    din("w2t", [96, 25, 128])
    din("w3t", [128, 25, 256])
    din("fc1wt", [1024, 2048])
    din("fc2wt", [2048, 2048])
    din("wp2", [2048, 2048])
    din("fc3wt", [128, 16, 10])
    din("b1", [96], F32)
    din("b2", [128], F32)
    din("b3", [256], F32)
    din("fb1", [2048], F32)
    din("fb2", [2048], F32)
    din("fb3", [10], F32)
    din("g", [BC, 2048], F32)
    din("noise", [BC, 2048], F32)
    din("zeros", [LEAD + BC * H1BLK + LEAD])
    out_d = nc.dram_tensor("out", [BC, 10], F32, kind="ExternalOutput").ap()
    import os
    dbg = {}
    if os.environ.get("DBG_DUMP", "0") == "1":
        for nm in ("h5", "h6eb", "z2", "s2", "pebs", "retain", "mask", "h_ed",
                   "h6"):
            dbg[nm] = nc.dram_tensor("dbg_" + nm, [BC, 2048], F32,
                                     kind="ExternalOutput").ap()
        dbg["h4"] = nc.dram_tensor("dbg_h4", [128, 8, 64], F32,
                                   kind="ExternalOutput").ap()

    tc = tile.TileContext(nc, num_cores=NCORES)
    with tc:
        with tc.tile_pool(name="perm", bufs=1) as perm:
            # persistent across reps: biases, identity
            ident = perm.tile([64, 64], F32)
            make_identity(nc, ident)
            b1 = perm.tile([96, 1], F32)
            nc.sync.dma_start(out=b1, in_=d["b1"].rearrange("(c o) -> c o", o=1))
            b2 = perm.tile([128, 1], F32)
            nc.sync.dma_start(out=b2, in_=d["b2"].rearrange("(c o) -> c o", o=1))
            b3 = perm.tile([256 // 2, 2], F32)  # [co%128, tile] per M-tile
            nc.sync.dma_start(
                out=b3, in_=d["b3"].rearrange("(t c) -> c t", t=2))
            # fc bias rows replicated to 64 partitions
            fb1 = perm.tile([BC, 2048], F32)
            nc.gpsimd.dma_start(out=fb1, in_=AP(
                tensor=d["fb1"].tensor, offset=0, ap=[[0, BC], [1, 2048]]))
            fb2 = perm.tile([BC, 2048], F32)
            nc.gpsimd.dma_start(out=fb2, in_=AP(
                tensor=d["fb2"].tensor, offset=0, ap=[[0, BC], [1, 2048]]))
            fb3 = perm.tile([BC, 10], F32)
            nc.gpsimd.dma_start(out=fb3, in_=AP(
                tensor=d["fb3"].tensor, offset=0, ap=[[0, BC], [1, 10]]))
            gt = perm.tile([BC, 2048], F32)
            nc.sync.dma_start(out=gt, in_=d["g"])
            noise = perm.tile([BC, 2048], F32)
            nc.sync.dma_start(out=noise, in_=d["noise"])

            for _ in range(rep):
                one_pass(nc, tc, d, out_d, ident, b1, b2, b3, fb1, fb2, fb3,
                         gt, noise, dbg, part)
    nc.finalize()
    return nc


def one_pass(nc, tc, d, out_d, ident, b1, b2, b3, fb1, fb2, fb3, gt, noise,
             dbg, part="all"):
    ctx_args = dict(nc=nc, tc=tc)

    with tc.tile_pool(name="acts", bufs=1) as acts:
        h3a = acts.tile([128, BC, 4], F32)
        h3b = acts.tile([128, BC, 4], F32)
        if part == "c2mm":
            with tc.tile_pool(name="xph1", bufs=1) as ph1:
                h1 = ph1.tile([96, LEAD + BC * H1BLK + LEAD], F32R)
                nc.gpsimd.dma_start(out=h1, in_=AP(
                    tensor=d["zeros"].tensor, offset=0,
                    ap=[[0, 96], [1, LEAD + BC * H1BLK + LEAD]]))
                with tc.tile_pool(name="xc2w", bufs=1) as wp, \
                     tc.tile_pool(name="xc2t", bufs=4) as tp, \
                     tc.tile_pool(name="xc2ps", bufs=2, space="PSUM") as pp:
                    w2 = wp.tile([96, 25, 128], F32R)
                    nc.sync.dma_start(out=w2, in_=d["w2t"])
                    for c in range(BC // 8):
                        ps = pp.tile([128, 8 * H1BLK], F32)
                        for k in range(25):
                            for g in range(4):
                                src = mkap(h1[:, :],
                                           LEAD + c * 8 * H1BLK
                                           + (g * 32 + OFFS[k][0]) * H1W
                                           + 2 + OFFS[k][1],
                                           [[H1W, 32], [1, 14]])
                                dst = mkap(ps[:, :], g * 32 * H1W + 2,
                                           [[H1W, 32], [1, 14]])
                                nc.tensor.matmul(dst, w2[:, k, :], src,
                                                 start=(k == 0), stop=(k == 24),
                                                 skip_group_check=True)
                        sc = tp.tile([128, 8 * H1BLK], F32, tag="sc")
                        nc.scalar.copy(sc[:, :], ps[:, :])
                ot = acts.tile([BC, 10], F32, tag="oc")
                nc.vector.tensor_copy(ot[:, :], sc[0:BC, 0:10])
                nc.sync.dma_start(out=out_d, in_=ot[:, :])
            return
        if part == "fc":
            nc.gpsimd.dma_start(out=h3a, in_=AP(
                tensor=d["zeros"].tensor, offset=0, ap=[[0, 128], [1, BC * 4]]))
            nc.gpsimd.dma_start(out=h3b, in_=AP(
                tensor=d["zeros"].tensor, offset=0, ap=[[0, 128], [1, BC * 4]]))
            fc_eb(d, out_d, h3a, h3b, ident, fb1, fb2, fb3, gt, noise, dbg,
                  **ctx_args)
            return
        with tc.tile_pool(name="ph2", bufs=1) as ph2:
            h2 = ph2.tile([128, LEAD + BC * H2BLK + LEAD], F32R)
            nc.gpsimd.dma_start(out=h2, in_=AP(
                tensor=d["zeros"].tensor, offset=0,
                ap=[[0, 128], [1, LEAD + BC * H2BLK + LEAD]]))
            with tc.tile_pool(name="ph1", bufs=1) as ph1:
                h1 = ph1.tile([96, LEAD + BC * H1BLK + LEAD], F32R)
                nc.gpsimd.dma_start(out=h1, in_=AP(
                    tensor=d["zeros"].tensor, offset=0,
                    ap=[[0, 96], [1, LEAD + BC * H1BLK + LEAD]]))
                conv1(d, h1, b1, **ctx_args)
                conv2(d, h1, h2, b2, **ctx_args)
            conv3(d, h2, b3, h3a, h3b, **ctx_args)
        if part == "conv":
            ot = acts.tile([BC, 10], F32, tag="oc")
            nc.vector.tensor_copy(ot[:, :4], h3a[0:BC, 0, :])
            nc.sync.dma_start(out=out_d, in_=ot[:, :])
            return
        fc_eb(d, out_d, h3a, h3b, ident, fb1, fb2, fb3, gt, noise, dbg,
              **ctx_args)


def conv1(d, h1, b1, nc, tc):
    """R [75, BC*900] @ W1T [75, 96] -> psum [96, 900/sample] -> pool+relu+bias
    -> h1 blocks."""
    CH = 8  # samples per R chunk
    with tc.tile_pool(name="c1w", bufs=1) as wp, \
         tc.tile_pool(name="c1r", bufs=2) as rp, \
         tc.tile_pool(name="c1t", bufs=4) as tp, \
         tc.tile_pool(name="c1ps", bufs=3, space="PSUM") as pp:
        w1 = wp.tile([75, 96], F32R)
        nc.sync.dma_start(out=w1, in_=d["w1t"])
        for c in range(BC // CH):
            r = rp.tile([75, CH * 900], F32R, tag="r")
            nc.sync.dma_start(out=r, in_=d["r1"][:, c * CH * 900:(c + 1) * CH * 900])
            for s in range(CH):
                n = c * CH + s
                ps = pp.tile([96, 1024], F32)
                nc.tensor.matmul(ps[:, 0:512], w1, r[:, s * 900: s * 900 + 512],
                                 start=True, stop=True)
                nc.tensor.matmul(ps[:, 512:900], w1,
                                 r[:, s * 900 + 512: s * 900 + 900],
                                 start=True, stop=True)
                # maxpool 3x3 s2 over 30x30 -> 14x14 (relu after pool)
                sc = tp.tile([96, 900], F32, tag="sc")
                nc.scalar.copy(sc[:, :], ps[:, 0:900])
                px = tp.tile([96, 30 * 14], F32, tag="px")
                nc.vector.tensor_tensor(
                    px[:, :], mkap(sc[:, :], 0, [[30, 30], [2, 14]]),
                    mkap(sc[:, :], 1, [[30, 30], [2, 14]]), AluOpType.max)
                nc.vector.tensor_tensor(
                    px[:, :], px[:, :],
                    mkap(sc[:, :], 2, [[30, 30], [2, 14]]), AluOpType.max)
                po = tp.tile([96, 196], F32, tag="po")
                nc.vector.tensor_tensor(
                    po[:, :], mkap(px[:, :], 0, [[28, 14], [1, 14]]),
                    mkap(px[:, :], 14, [[28, 14], [1, 14]]), AluOpType.max)
                nc.vector.tensor_tensor(
                    po[:, :], po[:, :],
                    mkap(px[:, :], 28, [[28, 14], [1, 14]]), AluOpType.max)
                # relu(pool + b) into h1 block (row/col 1..15)
                dst = mkap(h1[:, :], LEAD + n * H1BLK + H1W + 2,
                           [[H1W, 14], [1, 14]])
                nc.scalar.activation(dst, po[:, :],
                                     mybir.ActivationFunctionType.Relu,
                                     bias=b1[:, :])


def conv_shift(d, wname, src, dst_h, cin, cout, h_in, w_in, chunk, bias,
               out_pool_geom, nc, tc, name):
    """Shared conv2/conv3: shifted-offset matmuls over block-layout src,
    pool+relu+bias into dst blocks (or returned raw tiles for conv3)."""
    raise NotImplementedError


def conv2(d, h1, h2, b2, nc, tc):
    CH = 8                       # samples per psum chunk
    ROWS = CH * H1H              # 128 rows of 16
    with tc.tile_pool(name="c2w", bufs=1) as wp, \
         tc.tile_pool(name="c2t", bufs=4) as tp, \
         tc.tile_pool(name="c2ps", bufs=2, space="PSUM") as pp:
        w2 = wp.tile([96, 25, 128], F32R)
        nc.sync.dma_start(out=w2, in_=d["w2t"])
        for c in range(BC // CH):
            ps = pp.tile([128, CH * H1BLK], F32)  # 2048 cols = 4 banks
            for k, (dy, dx) in enumerate(OFFS):
                for g in range(ROWS // 32):
                    src = mkap(h1[:, :],
                               LEAD + c * CH * H1BLK + (g * 32 + dy) * H1W
                               + 2 + dx,
                               [[H1W, 32], [1, 14]])
                    dst = mkap(ps[:, :], g * 32 * H1W + 2,
                               [[H1W, 32], [1, 14]])
                    nc.tensor.matmul(dst, w2[:, k, :], src,
                                     start=(k == 0), stop=(k == 24),
                                     skip_group_check=True)
            # pool 14x14 -> 6x6 + relu + bias, per sample
            sc = tp.tile([128, CH * H1BLK], F32, tag="sc")
            nc.scalar.copy(sc[:, :], ps[:, :])
            for s in range(CH):
                n = c * CH + s
                base = s * H1BLK + H1W + 2  # row1,col2 of block = out (0,0)
                px = tp.tile([128, 14 * 6], F32, tag="px")
                nc.vector.tensor_tensor(
                    px[:, :], mkap(sc[:, :], base, [[H1W, 14], [2, 6]]),
                    mkap(sc[:, :], base + 1, [[H1W, 14], [2, 6]]), AluOpType.max)
                nc.vector.tensor_tensor(
                    px[:, :], px[:, :],
                    mkap(sc[:, :], base + 2, [[H1W, 14], [2, 6]]), AluOpType.max)
                po = tp.tile([128, 36], F32, tag="po")
                nc.vector.tensor_tensor(
                    po[:, :], mkap(px[:, :], 0, [[12, 6], [1, 6]]),
                    mkap(px[:, :], 6, [[12, 6], [1, 6]]), AluOpType.max)
                nc.vector.tensor_tensor(
                    po[:, :], po[:, :],
                    mkap(px[:, :], 12, [[12, 6], [1, 6]]), AluOpType.max)
                dst = mkap(h2[:, :], LEAD + n * H2BLK + H2W + 2,
                           [[H2W, 6], [1, 6]])
                nc.scalar.activation(dst, po[:, :],
                                     mybir.ActivationFunctionType.Relu,
                                     bias=b2[:, :])


def conv3(d, h2, b3, h3a, h3b, nc, tc):
    CH = 16                      # samples per psum chunk
    ROWS = CH * H2H              # 128 rows of 8
    with tc.tile_pool(name="c3w", bufs=1) as wp, \
         tc.tile_pool(name="c3t", bufs=4) as tp, \
         tc.tile_pool(name="c3ps", bufs=4, space="PSUM") as pp:
        w3 = wp.tile([128, 25, 256], F32R)
        nc.sync.dma_start(out=w3, in_=d["w3t"])
        for c in range(BC // CH):
            for mt, h3 in ((0, h3a), (1, h3b)):
                ps = pp.tile([128, CH * H2BLK], F32)  # 1024 cols = 2 banks
                for k, (dy, dx) in enumerate(OFFS):
                    for g in range(ROWS // 64):
                        src = mkap(h2[:, :],
                                   LEAD + c * CH * H2BLK + (g * 64 + dy) * H2W
                                   + 2 + dx,
                                   [[H2W, 64], [1, 6]])
                        dst = mkap(ps[:, :], g * 64 * H2W + 2,
                                   [[H2W, 64], [1, 6]])
                        nc.tensor.matmul(dst, w3[:, k, 128 * mt:128 * mt + 128],
                                         src, start=(k == 0), stop=(k == 24),
                                         skip_group_check=True)
                sc = tp.tile([128, CH * H2BLK], F32, tag="sc")
                nc.scalar.copy(sc[:, :], ps[:, :])
                for s in range(CH):
                    n = c * CH + s
                    base = s * H2BLK + H2W + 2
                    px = tp.tile([128, 6 * 2], F32, tag="px")
                    nc.vector.tensor_tensor(
                        px[:, :], mkap(sc[:, :], base, [[H2W, 6], [2, 2]]),
                        mkap(sc[:, :], base + 1, [[H2W, 6], [2, 2]]),
                        AluOpType.max)
                    nc.vector.tensor_tensor(
                        px[:, :], px[:, :],
                        mkap(sc[:, :], base + 2, [[H2W, 6], [2, 2]]),
                        AluOpType.max)
                    po = tp.tile([128, 4], F32, tag="po")
                    nc.vector.tensor_tensor(
                        po[:, :], mkap(px[:, :], 0, [[4, 2], [1, 2]]),
                        mkap(px[:, :], 2, [[4, 2], [1, 2]]), AluOpType.max)
                    nc.vector.tensor_tensor(
                        po[:, :], po[:, :],
                        mkap(px[:, :], 4, [[4, 2], [1, 2]]), AluOpType.max)
                    nc.scalar.activation(po[:, :], po[:, :],
                                         mybir.ActivationFunctionType.Relu,
                                         bias=b3[:, mt:mt + 1])
                    nc.vector.tensor_copy(h3[:, n, :], po[:, :])


def fc_eb(d, out_d, h3a, h3b, ident, fb1, fb2, fb3, gt, noise, dbg, nc, tc):
    def dump(nm, ap):
        if dbg:
            nc.sync.dma_start(out=dbg[nm], in_=ap)
    Relu = mybir.ActivationFunctionType.Relu
    Copy = mybir.ActivationFunctionType.Copy
    with tc.tile_pool(name="fca", bufs=1) as fa, \
         tc.tile_pool(name="wstream", bufs=3) as ws, \
         tc.tile_pool(name="wrelu", bufs=2) as wr, \
         tc.tile_pool(name="fct", bufs=1) as ft:

        # ---- h4T [128, 8, 64]: ci = co*4 + s, from h3a/h3b via DMA reshape
        h4t = fa.tile([128, 8, 64], F32R)
        if dbg:
            h4f = fa.tile([128, 8, 64], F32)
        for t in range(8):
            h3 = h3a if t < 4 else h3b
            co0 = 32 * (t % 4)
            for sp in range(4):
                dst = h4t[sp::4, t, :]
                src = h3[co0:co0 + 32, :, sp]
                nc.gpsimd.dma_start(out=dst, in_=src)
        if dbg:
            nc.vector.tensor_copy(h4f[:, :, :], h4t[:, :, :].bitcast(F32))
            nc.sync.dma_start(out=dbg["h4"], in_=h4f[:, :, :])

        # ---- fc1: h5 = relu(h4 @ fc1_w.T + fb1)
        h5 = fa.tile([BC, 2048], F32)
        h5t = fa.tile([128, 16, 64], F32R)
        with tc.tile_pool(name="ps1", bufs=1, space="PSUM") as pp, \
             tc.tile_pool(name="pstr", bufs=4, space="PSUM") as ptr:
            ps = pp.tile([BC, 2048], F32)
            for kt in range(8):
                w = ws.tile([128, 2048], F32R, tag="w")
                nc.sync.dma_start(out=w, in_=d["fc1wt"][128 * kt:128 * kt + 128, :])
                for ch in range(4):
                    nc.tensor.matmul(ps[:, 512 * ch:512 * ch + 512],
                                     h4t[:, kt, :], w[:, 512 * ch:512 * ch + 512],
                                     start=(kt == 0), stop=(kt == 7))
            tmp = ft.tile([BC, 2048], F32, tag="ta")
            nc.vector.tensor_tensor(tmp, ps[:, :], fb1[:, :], AluOpType.add)
            nc.scalar.activation(h5[:, :], tmp[:, :], Relu)
            dump("h5", h5[:, :])
            # h5T via PE transposes
            for kt in range(16):
                pt = ptr.tile([128, 64], F32, tag="tr")
                nc.tensor.transpose(pt[:, :], h5[:, 128 * kt:128 * kt + 128],
                                    ident[:, :])
                nc.scalar.copy(h5t[:, kt, :], pt[:, :])

        # ---- pass1: h6_eb = relu(h5 @ fc2_w.T + fb2); z2 = h5 @ Wp2.T
        h6eb = fa.tile([BC, 2048], F32)
        s2 = fa.tile([BC, 2048], F32)
        with tc.tile_pool(name="ps2", bufs=2, space="PSUM") as pp:
            pa = pp.tile([BC, 2048], F32, tag="big")
            pz = pp.tile([BC, 2048], F32, tag="big")
            for kt in range(16):
                w = ws.tile([128, 2048], F32R, tag="w")
                nc.sync.dma_start(out=w, in_=d["fc2wt"][128 * kt:128 * kt + 128, :])
                wp = wr.tile([128, 2048], F32R, tag="wp")
                nc.scalar.activation(wp[:, :], w[:, :], Relu)
                for ch in range(4):
                    sl = slice(512 * ch, 512 * ch + 512)
                    nc.tensor.matmul(pa[:, sl], h5t[:, kt, :], w[:, sl],
                                     start=(kt == 0), stop=(kt == 15))
                    nc.tensor.matmul(pz[:, sl], h5t[:, kt, :], wp[:, sl],
                                     start=(kt == 0), stop=(kt == 15))
            tmp = ft.tile([BC, 2048], F32, tag="ta")
            nc.vector.tensor_tensor(tmp, pa[:, :], fb2[:, :], AluOpType.add)
            nc.scalar.activation(h6eb[:, :], tmp[:, :], Relu)
            dump("h6eb", h6eb[:, :])
            # EB through fc3: p_h6 = h6eb * g / z3 (z3 = rowsum(h6eb*g))
            hg = ft.tile([BC, 2048], F32, tag="tb")
            nc.vector.tensor_tensor(hg, h6eb[:, :], gt[:, :], AluOpType.mult)
            z3 = ft.tile([BC, 1], F32, tag="z3")
            nc.vector.tensor_reduce(z3, hg[:, :], mybir.AxisListType.X,
                                    AluOpType.add)
            z3c = ft.tile([BC, 1], F32, tag="z3c")
            nc.vector.tensor_scalar_max(z3c, z3[:, :], 1e-30)
            rz3 = ft.tile([BC, 1], F32, tag="rz3")
            nc.vector.reciprocal(rz3, z3c[:, :])
            gt3 = ft.tile([BC, 1], F32, tag="gt3")
            nc.vector.tensor_scalar(gt3, z3[:, :], 0.0, None, AluOpType.is_gt)
            rz3m = ft.tile([BC, 1], F32, tag="rz3m")
            nc.vector.tensor_tensor(rz3m, rz3[:, :], gt3[:, :], AluOpType.mult)
            ph6 = ft.tile([BC, 2048], F32, tag="ta")
            nc.vector.tensor_scalar_mul(ph6, hg[:, :], rz3m[:, :])
            # s2 = where(z2>0, p_h6/z2, 0)
            z2c = ft.tile([BC, 2048], F32, tag="tc")
            nc.vector.tensor_scalar_max(z2c, pz[:, :], 1e-30)
            rz2 = ft.tile([BC, 2048], F32, tag="td")
            nc.vector.reciprocal(rz2, z2c[:, :])
            gt2 = ft.tile([BC, 2048], F32, tag="tc")
            nc.vector.tensor_scalar(gt2, pz[:, :], 0.0, None, AluOpType.is_gt)
            pr = ft.tile([BC, 2048], F32, tag="tb")
            nc.vector.tensor_tensor(pr, ph6[:, :], rz2[:, :], AluOpType.mult)
            nc.vector.tensor_tensor(s2[:, :], pr[:, :], gt2[:, :], AluOpType.mult)
            if dbg:
                zc = ft.tile([BC, 2048], F32, tag="td")
                nc.vector.tensor_copy(zc, pz[:, :])
                dump("z2", zc[:, :])
                dump("s2", s2[:, :])

        # ---- s2T, pass2: r = s2 @ Wp2 ; pebs = h5 * r ; h_ed
        s2t = fa.tile([128, 16, 64], F32R)
        with tc.tile_pool(name="pstr2", bufs=4, space="PSUM") as ptr:
            for kt in range(16):
                pt = ptr.tile([128, 64], F32, tag="tr")
                nc.tensor.transpose(pt[:, :], s2[:, 128 * kt:128 * kt + 128],
                                    ident[:, :])
                nc.scalar.copy(s2t[:, kt, :], pt[:, :])
        h_ed = fa.tile([BC, 2048], F32)
        with tc.tile_pool(name="ps3", bufs=1, space="PSUM") as pp:
            ps = pp.tile([BC, 2048], F32)
            for kt in range(16):
                w = ws.tile([128, 2048], F32R, tag="w")
                nc.sync.dma_start(out=w, in_=d["wp2"][128 * kt:128 * kt + 128, :])
                for ch in range(4):
                    sl = slice(512 * ch, 512 * ch + 512)
                    nc.tensor.matmul(ps[:, sl], s2t[:, kt, :], w[:, sl],
                                     start=(kt == 0), stop=(kt == 15))
            pebs = ft.tile([BC, 2048], F32, tag="ta")
            nc.vector.tensor_tensor(pebs, h5[:, :], ps[:, :], AluOpType.mult)
            # retain_p = (0.5 - 0.5*pebs) / (1023*pebs + 0.5)
            dn = ft.tile([BC, 2048], F32, tag="tb")
            nc.vector.tensor_scalar(dn, pebs[:, :], 1023.0, 0.5,
                                    AluOpType.mult, AluOpType.add)
            rd = ft.tile([BC, 2048], F32, tag="tc")
            nc.vector.reciprocal(rd, dn[:, :])
            t05 = ft.tile([BC, 2048], F32, tag="tb")
            nc.vector.tensor_scalar(t05, pebs[:, :], -0.5, 0.5,
                                    AluOpType.mult, AluOpType.add)
            retain = ft.tile([BC, 2048], F32, tag="td")
            nc.vector.tensor_tensor(retain, t05[:, :], rd[:, :], AluOpType.mult)
            dump("pebs", pebs[:, :])
            dump("retain", retain[:, :])
            mask = ft.tile([BC, 2048], F32, tag="tb")
            nc.vector.tensor_tensor(mask, noise[:, :], retain[:, :],
                                    AluOpType.is_lt)
            rc = ft.tile([BC, 2048], F32, tag="tc")
            nc.vector.tensor_scalar_max(rc, retain[:, :], 1e-30)
            rr = ft.tile([BC, 2048], F32, tag="ta")
            nc.vector.reciprocal(rr, rc[:, :])
            hm = ft.tile([BC, 2048], F32, tag="tc")
            nc.vector.tensor_tensor(hm, h5[:, :], mask[:, :], AluOpType.mult)
            nc.vector.tensor_tensor(h_ed[:, :], hm[:, :], rr[:, :],
                                    AluOpType.mult)
            dump("mask", mask[:, :])
            dump("h_ed", h_ed[:, :])

        # ---- h_edT, pass3: h6 = relu(h_ed @ fc2_w.T + fb2)
        hedt = fa.tile([128, 16, 64], F32R)
        with tc.tile_pool(name="pstr3", bufs=4, space="PSUM") as ptr:
            for kt in range(16):
                pt = ptr.tile([128, 64], F32, tag="tr")
                nc.tensor.transpose(pt[:, :], h_ed[:, 128 * kt:128 * kt + 128],
                                    ident[:, :])
                nc.scalar.copy(hedt[:, kt, :], pt[:, :])
        h6 = fa.tile([BC, 2048], F32)
        with tc.tile_pool(name="ps4", bufs=1, space="PSUM") as pp:
            ps = pp.tile([BC, 2048], F32)
            for kt in range(16):
                w = ws.tile([128, 2048], F32R, tag="w")
                nc.sync.dma_start(out=w, in_=d["fc2wt"][128 * kt:128 * kt + 128, :])
                for ch in range(4):
                    sl = slice(512 * ch, 512 * ch + 512)
                    nc.tensor.matmul(ps[:, sl], hedt[:, kt, :], w[:, sl],
                                     start=(kt == 0), stop=(kt == 15))
            tmp = ft.tile([BC, 2048], F32, tag="ta")
            nc.vector.tensor_tensor(tmp, ps[:, :], fb2[:, :], AluOpType.add)
            nc.scalar.activation(h6[:, :], tmp[:, :], Relu)
            dump("h6", h6[:, :])

        # ---- out = h6 @ fc3_w.T + fb3
        h6t = fa.tile([128, 16, 64], F32R)
        with tc.tile_pool(name="pstr4", bufs=4, space="PSUM") as ptr, \
             tc.tile_pool(name="ps5", bufs=1, space="PSUM") as pp:
            for kt in range(16):
                pt = ptr.tile([128, 64], F32, tag="tr")
                nc.tensor.transpose(pt[:, :], h6[:, 128 * kt:128 * kt + 128],
                                    ident[:, :])
                nc.scalar.copy(h6t[:, kt, :], pt[:, :])
            w3t = fa.tile([128, 16, 10], F32R)
            nc.sync.dma_start(out=w3t, in_=d["fc3wt"])
            po = pp.tile([BC, 10], F32)
            for kt in range(16):
                nc.tensor.matmul(po[:, :], h6t[:, kt, :], w3t[:, kt, :],
                                 start=(kt == 0), stop=(kt == 15))
            ot = ft.tile([BC, 10], F32, tag="ot")
            nc.vector.tensor_tensor(ot, po[:, :], fb3[:, :], AluOpType.add)
            nc.sync.dma_start(out=out_d, in_=ot[:, :])


# ---------------------------------------------------------------- host side

def prep_host(inputs):
    x = np.asarray(inputs["x"], np.float32)            # [512, 3, 32, 32]
    noise = np.asarray(inputs["noise"], np.float32)
    label = np.asarray(inputs["label"]).astype(np.int64)

    # conv1 im2col: R[p=(ky*5+kx)*3+ci, n, y*30+x] = xpad[n, ci, y+ky, x+kx]
    B = x.shape[0]
    xpad = np.zeros((B, 3, 34, 34), np.float32)
    xpad[:, :, 1:33, 1:33] = x
    win = np.lib.stride_tricks.sliding_window_view(
        xpad, (5, 5), axis=(2, 3))                     # [B, 3, 30, 30, 5, 5]
    # -> [ky, kx, ci, n, y, x]
    R = win.transpose(4, 5, 1, 0, 2, 3).reshape(75, B, 900)

    w1t = np.ascontiguousarray(
        np.asarray(inputs["conv1_w"], np.float32)
        .transpose(2, 3, 1, 0).reshape(75, 96))
    w2t = np.ascontiguousarray(
        np.asarray(inputs["conv2_w"], np.float32)
        .transpose(1, 2, 3, 0).reshape(96, 25, 128))
    w3t = np.ascontiguousarray(
        np.asarray(inputs["conv3_w"], np.float32)
        .transpose(1, 2, 3, 0).reshape(128, 25, 256))
    fc1wt = np.ascontiguousarray(np.asarray(inputs["fc1_w"], np.float32).T)
    fc2wt = np.ascontiguousarray(np.asarray(inputs["fc2_w"], np.float32).T)
    wp2 = np.maximum(np.asarray(inputs["fc2_w"], np.float32), 0.0)
    fc3wt = np.ascontiguousarray(
        np.asarray(inputs["fc3_w"], np.float32).T.reshape(16, 128, 10)
        .transpose(1, 0, 2))
    g_all = np.maximum(np.asarray(inputs["fc3_w"], np.float32), 0.0)[label]

    shared = dict(
        w1t=w1t, w2t=w2t, w3t=w3t, fc1wt=fc1wt, fc2wt=fc2wt, wp2=wp2,
        fc3wt=fc3wt,
        b1=np.asarray(inputs["conv1_b"], np.float32),
        b2=np.asarray(inputs["conv2_b"], np.float32),
        b3=np.asarray(inputs["conv3_b"], np.float32),
        fb1=np.asarray(inputs["fc1_b"], np.float32),
        fb2=np.asarray(inputs["fc2_b"], np.float32),
        fb3=np.asarray(inputs["fc3_b"], np.float32),
    )
    in_maps = []
    for c in range(NCORES):
        s = slice(c * BC, (c + 1) * BC)
        m = dict(shared)
        m["r1"] = np.ascontiguousarray(R[:, s, :]).reshape(75, BC * 900)
        m["noise"] = np.ascontiguousarray(noise[s])
        m["g"] = np.ascontiguousarray(g_all[s])
        m["zeros"] = np.zeros(LEAD + BC * H1BLK + LEAD, np.float32)
        in_maps.append(m)
    return in_maps


_CACHED = {}


def kernel(**inputs):
    rep = int(_CACHED.get("rep", 1))
    key = ("nc", rep)
    if key not in _CACHED:
        _CACHED[key] = build_bass(rep)
    nc = _CACHED[key]
    in_maps = prep_host(inputs)
    res = run_bass_kernel_spmd(nc, in_maps, core_ids=list(range(NCORES)))
    out = np.concatenate([r["out"] for r in res.results], axis=0)
    return out.astype(np.float32)


if __name__ == "__main__":
    import reference
    ins = {k: np.asarray(v) for k, v in reference.setup_inputs().items()}
    exp = np.asarray(reference.reference(**ins))
    act = kernel(**ins)
    rel = np.linalg.norm(act - exp) / np.linalg.norm(exp)
    print("Relative error:", rel)
